# revision 1
# baseline (speedup 1.0000x reference)
"""Trainium2 Bass kernel: Encoder_HieStackedCorr (UnCorrVmat_Detail, t_method='uncorr').

Math (per batch b):
    W1 = wn(U1_v, U1_g); W2 = wn(U2_v, U2_g)
    R = relu(V @ W1.T + b1)          [N, LR]
    L = relu(V @ W2.T + b2)          [N, LR]
    UnCorr = L @ R.T                 [N, N]
    d[n] = UnCorr[n, n] = sum_l L[n,l] R[n,l]
    dr = 1/sqrt(d + eps)
    A = 1 + I - dr dr^T * UnCorr
    v = mean_n (A @ V) = (1/N) * s @ V  where s[m] = N + 1 - dr[m] * (t . R[m,:]),
                                              t = sum_n dr[n] L[n,:]
    feat = v @ W_lin.T + b_lin ; out = batchnorm(feat)   (training-mode stats)

The N x N matrix is never materialized: mean-pooling commutes with the matmul,
collapsing the O(B N^2 (LR+D)) reference into O(B N D LR) work.

Sharding: data-parallel over batch, 4 batches per core on 8 cores.  Each core
computes v for its 4 batches; the tiny [32,256] linear + batchnorm epilogue
(needs cross-core batch stats) runs on host.
"""

import os
import sys

import numpy as np

for _p in ("/opt/trn_rl_repo", "/root/.axon_site/_ro/trn_rl_repo"):
    if os.path.isdir(_p) and _p not in sys.path:
        sys.path.insert(0, _p)
        break

import ml_dtypes  # noqa: E402
import concourse.bass as bass  # noqa: E402
import concourse.bacc as bacc  # noqa: E402
import concourse.mybir as mybir  # noqa: E402
import concourse.tile as tile  # noqa: E402
from concourse.bass_utils import run_bass_kernel_spmd  # noqa: E402


def _ensure_ntff_hook():
    """Shim the missing ``antenv.axon_hooks`` registry so trace=True works.

    The agent image's ``antenv`` lacks ``axon_hooks``; the ctypes NTFF driver
    in ``trn_agent_boot.trn_boot`` is present and the injected libaxon_pjrt.so
    exports the profile symbols, so wire them together here.
    """
    import types

    try:
        from antenv.axon_hooks import get_axon_ntff_profile_hook  # noqa: F401
        return
    except ImportError:
        pass
    try:
        from trn_agent_boot.trn_boot import _ntff_profile_via_ctypes
        hook = _ntff_profile_via_ctypes("/opt/axon/libaxon_pjrt.so")
    except Exception:
        hook = None
    mod = types.ModuleType("antenv.axon_hooks")
    mod._hook = hook
    mod.get_axon_ntff_profile_hook = lambda: mod._hook
    mod.set_axon_ntff_profile_hook = lambda h: setattr(mod, "_hook", h)
    sys.modules["antenv.axon_hooks"] = mod


_ensure_ntff_hook()

# Problem constants (hardcoded; see module docstring).
B, N, D, LR, EMB = 32, 2048, 256, 64, 256
NCORES = 8
B_LOC = B // NCORES          # 4 batches per core
ROWS = B_LOC * N             # 8192 rows per core
NT_B = N // 128              # 16 row-tiles per batch
NBLK = N // 512              # 4 512-row blocks per batch
EPS_DIAG = 1e-6
EPS_BN = 1e-5

F32 = mybir.dt.float32
BF16 = mybir.dt.bfloat16

# dt: dtype for V/Vt/W, the L/R activations and every matmul operand
# ("f32" = exact but 4 cyc/row on the PE, "bf16" = 1 cyc/row).
CONFIG = dict(dt="f32", trace=False)

_CACHE = {}


def _build(cfg):
    DT = BF16 if cfg["dt"] == "bf16" else F32
    nc = bacc.Bacc("TRN2", target_bir_lowering=False, debug=False)

    v_d = nc.dram_tensor("v", [ROWS, D], DT, kind="ExternalInput").ap()
    vt_d = nc.dram_tensor("vt", [2, 128, ROWS], DT, kind="ExternalInput").ap()
    w1_d = nc.dram_tensor("w1t", [2, 128, LR], DT, kind="ExternalInput").ap()
    w2_d = nc.dram_tensor("w2t", [2, 128, LR], DT, kind="ExternalInput").ap()
    b1_d = nc.dram_tensor("b1", [LR, 1], F32, kind="ExternalInput").ap()
    b2_d = nc.dram_tensor("b2", [LR, 1], F32, kind="ExternalInput").ap()
    out_d = nc.dram_tensor("vmean", [1, B_LOC * D], F32, kind="ExternalOutput").ap()

    with tile.TileContext(nc) as tc:
        with (
            tc.tile_pool(name="const", bufs=1) as cpool,
            tc.tile_pool(name="vst", bufs=1) as vpool,
            tc.tile_pool(name="lrbuf", bufs=2) as lrpool,
            tc.tile_pool(name="blk", bufs=3) as bpool,
            tc.tile_pool(name="rows", bufs=2) as rpool,
            tc.tile_pool(name="ps_lr", bufs=2, space="PSUM") as ps_lr,
            tc.tile_pool(name="ps_d", bufs=1, space="PSUM") as ps_d,
            tc.tile_pool(name="ps_misc", bufs=1, space="PSUM") as ps_misc,
            tc.tile_pool(name="dram", bufs=2, space="DRAM") as dpool,
        ):
            # ---- constants / weights ----
            w1_sb = cpool.tile([128, 2 * LR], DT)
            w2_sb = cpool.tile([128, 2 * LR], DT)
            nc.sync.dma_start(
                w1_sb[:].rearrange("p (c l) -> p c l", c=2),
                w1_d.rearrange("c p l -> p c l"),
            )
            nc.sync.dma_start(
                w2_sb[:].rearrange("p (c l) -> p c l", c=2),
                w2_d.rearrange("c p l -> p c l"),
            )
            b1_sb = cpool.tile([LR, 1], F32)
            b2_sb = cpool.tile([LR, 1], F32)
            nc.sync.dma_start(b1_sb[:], b1_d[:])
            nc.sync.dma_start(b2_sb[:], b2_d[:])
            ones64 = cpool.tile([LR, 1], DT)
            nc.vector.memset(ones64[:], 1.0)
            ones_k1 = cpool.tile([1, LR], DT)
            nc.vector.memset(ones_k1[:], 1.0)
            eps_sb = cpool.tile([1, 1], F32)
            nc.vector.memset(eps_sb[:], EPS_DIAG)

            out_sb = cpool.tile([1, B_LOC * D], F32)

            # per-batch persistent tiles, double buffered across batches
            v_t = {}
            vt_t = {}
            for b in range(B_LOC):
                # natural V for this batch: tile j at cols [j*D, (j+1)*D)
                v_t[b] = vpool.tile([128, NT_B * D], DT, tag="vnat", name=f"vnat{b}")
                # transposed V, both d-chunks: chunk c at cols [c*N, (c+1)*N)
                vt_t[b] = vpool.tile([128, 2 * N], DT, tag="vt", name=f"vt{b}")
                src = v_d.rearrange("(t p) d -> p t d", p=128)
                nc.sync.dma_start(
                    v_t[b][:].rearrange("p (t d) -> p t d", t=NT_B),
                    src[:, b * NT_B:(b + 1) * NT_B, :],
                )
                nc.sync.dma_start(
                    vt_t[b][:].rearrange("p (c n) -> p c n", c=2),
                    vt_d[:, :, b * N:(b + 1) * N].rearrange("c p n -> p c n"),
                )

            for b in range(B_LOC):
                L_sb = lrpool.tile([LR, N], DT, tag="L")
                R_sb = lrpool.tile([LR, N], DT, tag="R")
                sq_row = rpool.tile([1, N], F32, tag="sq")     # sqrt(d + eps)
                dr_row = rpool.tile([1, N], F32, tag="dr")     # 1/sqrt(d + eps)
                s_row = rpool.tile([1, N], F32, tag="s")       # ((N+1) - c)/N
                for blk in range(NBLK):
                    f0 = blk * 512
                    # L/R = V @ W.T in transposed layout [LR, n-block]
                    L_ps = ps_lr.tile([LR, 512], F32, tag="Lps")
                    R_ps = ps_lr.tile([LR, 512], F32, tag="Rps")
                    for c in range(2):
                        rhs = vt_t[b][:, c * N + f0:c * N + f0 + 512]
                        nc.tensor.matmul(
                            L_ps[:], w2_sb[:, c * LR:(c + 1) * LR], rhs,
                            start=(c == 0), stop=(c == 1),
                        )
                        nc.tensor.matmul(
                            R_ps[:], w1_sb[:, c * LR:(c + 1) * LR], rhs,
                            start=(c == 0), stop=(c == 1),
                        )
                    # relu(+bias): R on ACT, L on DVE (balance engines)
                    nc.scalar.activation(
                        R_sb[:, f0:f0 + 512], R_ps[:],
                        mybir.ActivationFunctionType.Relu, bias=b1_sb[:], scale=1.0,
                    )
                    nc.vector.tensor_scalar(
                        L_sb[:, f0:f0 + 512], L_ps[:], b2_sb[:], 0.0,
                        mybir.AluOpType.add, mybir.AluOpType.max,
                    )
                    # diag: d[n] = sum_l L[l,n]*R[l,n] -> [1,512] via ones-matmul
                    prod = bpool.tile([LR, 512], DT, tag="prod")
                    nc.vector.tensor_tensor(
                        prod[:], L_sb[:, f0:f0 + 512], R_sb[:, f0:f0 + 512],
                        mybir.AluOpType.mult,
                    )
                    d_ps = ps_d.tile([1, 512], F32, tag="dps")
                    nc.tensor.matmul(
                        d_ps[:], ones64[:], prod[:],
                        start=True, stop=True,
                    )
                    # dr = 1/sqrt(d + eps)  (Rsqrt is banned on ACT; DVE reciprocal)
                    nc.scalar.activation(
                        sq_row[:, f0:f0 + 512], d_ps[:],
                        mybir.ActivationFunctionType.Sqrt, bias=eps_sb[:], scale=1.0,
                    )
                    nc.vector.reciprocal(
                        dr_row[:, f0:f0 + 512], sq_row[:, f0:f0 + 512]
                    )

                # t = sum_n dr[n] * L[n,:]   (chained fused multiply-reduce)
                dr_dt = dr_row
                if DT != F32:
                    dr_dt = rpool.tile([1, N], DT, tag="dr_dt", name=f"drdt{b}")
                    nc.scalar.activation(
                        dr_dt[:], dr_row[:], mybir.ActivationFunctionType.Copy
                    )
                ldr = lrpool.tile([LR, N], DT, tag="ldr", name=f"ldr{b}")
                for blk in range(NBLK):
                    f0 = blk * 512
                    rep_ps = ps_misc.tile([LR, 512], F32, tag="rep")
                    nc.tensor.matmul(
                        rep_ps[:], ones_k1[:], dr_dt[:, f0:f0 + 512],
                        start=True, stop=True,
                    )
                    nc.vector.tensor_tensor(
                        ldr[:, f0:f0 + 512], L_sb[:, f0:f0 + 512], rep_ps[:],
                        mybir.AluOpType.mult,
                    )
                t_sb = bpool.tile([LR, 1], F32, tag="t", name=f"tacc{b}")
                nc.vector.tensor_reduce(
                    t_sb[:], ldr[:], mybir.AxisListType.X, mybir.AluOpType.add,
                )
                t_dt = t_sb
                if DT != F32:
                    t_dt = bpool.tile([LR, 1], DT, tag="t_dt", name=f"tdt{b}")
                    nc.scalar.activation(
                        t_dt[:], t_sb[:], mybir.ActivationFunctionType.Copy
                    )

                # u = t . R[m,:] -> [1,512] blocks; c = u / sq; s = ((N+1)-c)/N
                for blk in range(NBLK):
                    f0 = blk * 512
                    u_ps = ps_misc.tile([1, 512], F32, tag="ups")
                    nc.tensor.matmul(
                        u_ps[:], t_dt[:], R_sb[:, f0:f0 + 512],
                        start=True, stop=True,
                    )
                    c_row = bpool.tile([1, 512], F32, tag="crow")
                    nc.vector.tensor_tensor(
                        c_row[:], u_ps[:], dr_row[:, f0:f0 + 512],
                        mybir.AluOpType.mult,
                    )
                    nc.scalar.activation(
                        s_row[:, f0:f0 + 512], c_row[:],
                        mybir.ActivationFunctionType.Copy,
                        bias=float(N + 1) / N, scale=-1.0 / N,
                    )

                # scatter s to partitions: s_col[p, j] = s[j*128 + p].
                # A direct SBUF->SBUF rearrange is NOT usable: the source AP's
                # first dim is interpreted as physical partitions by the DMA
                # descriptor generator (HW reads partitions 1.. as garbage).
                # Bounce through DRAM, where APs are plain strided views.
                s_dram = dpool.tile([1, N], F32, tag="sdram", name=f"sdram{b}")
                nc.sync.dma_start(s_dram[:], s_row[:])
                s_col = bpool.tile([128, NT_B], F32, tag="scol")
                nc.sync.dma_start(
                    s_col[:], s_dram.rearrange("a (j p) -> (a p) j", p=128)
                )
                s_dt = s_col
                if DT != F32:
                    s_dt = bpool.tile([128, NT_B], DT, tag="scol_dt")
                    nc.scalar.activation(
                        s_dt[:], s_col[:], mybir.ActivationFunctionType.Copy
                    )

                # v_mean = s^T @ V  (accumulate over the 16 row-tiles)
                v_ps = ps_misc.tile([1, D], F32, tag="vps")
                for j in range(NT_B):
                    nc.tensor.matmul(
                        v_ps[:], s_dt[:, j:j + 1],
                        v_t[b][:, j * D:(j + 1) * D],
                        start=(j == 0), stop=(j == NT_B - 1),
                    )
                nc.scalar.activation(
                    out_sb[:, b * D:(b + 1) * D], v_ps[:],
                    mybir.ActivationFunctionType.Copy,
                )

            nc.sync.dma_start(out_d[:], out_sb[:])

    nc.compile()
    return nc


def _host_prep(inputs, cfg):
    """Weight-norm, transposes, casts; returns per-core input maps + epilogue data."""
    np_dt = ml_dtypes.bfloat16 if cfg["dt"] == "bf16" else np.float32

    def wn(v, g):
        return v * (g / np.linalg.norm(v.astype(np.float64), axis=1)).astype(
            np.float32
        )[:, None]

    W1 = wn(np.asarray(inputs["U1_v"], np.float32), np.asarray(inputs["U1_g"], np.float32))
    W2 = wn(np.asarray(inputs["U2_v"], np.float32), np.asarray(inputs["U2_g"], np.float32))
    w1t = np.ascontiguousarray(W1.T).reshape(2, 128, LR).astype(np_dt)
    w2t = np.ascontiguousarray(W2.T).reshape(2, 128, LR).astype(np_dt)
    b1 = np.asarray(inputs["U1_b"], np.float32).reshape(LR, 1)
    b2 = np.asarray(inputs["U2_b"], np.float32).reshape(LR, 1)

    V = np.asarray(inputs["Vmat"], np.float32)  # [B, N, D]
    in_maps = []
    for k in range(NCORES):
        Vk = np.ascontiguousarray(V[k * B_LOC:(k + 1) * B_LOC].reshape(ROWS, D))
        vt = np.ascontiguousarray(Vk.T).reshape(2, 128, ROWS).astype(np_dt)
        in_maps.append({
            "v": Vk.astype(np_dt),
            "vt": vt,
            "w1t": w1t,
            "w2t": w2t,
            "b1": b1,
            "b2": b2,
        })
    return in_maps


def _epilogue(v_mean, inputs):
    """feat = v_mean @ W_lin.T + b_lin, then training-mode batchnorm."""
    W_lin = np.asarray(inputs["W_lin"], np.float32)
    b_lin = np.asarray(inputs["b_lin"], np.float32)
    gamma = np.asarray(inputs["gamma"], np.float32)
    beta = np.asarray(inputs["beta"], np.float32)
    feat = v_mean.astype(np.float32) @ W_lin.T + b_lin
    mu = feat.mean(axis=0)
    var = feat.var(axis=0)
    out = (feat - mu) / np.sqrt(var + EPS_BN) * gamma + beta
    return out.astype(np.float32)


def kernel(**inputs):
    cfg = dict(CONFIG)
    key = (cfg["dt"],)
    if key not in _CACHE:
        _CACHE[key] = _build(cfg)
    nc = _CACHE[key]
    in_maps = _host_prep(inputs, cfg)
    res = run_bass_kernel_spmd(
        nc, in_maps, core_ids=list(range(NCORES)), trace=cfg["trace"]
    )
    kernel.last_results = res
    v_mean = np.concatenate(
        [res.results[k]["vmean"].reshape(B_LOC, D) for k in range(NCORES)], axis=0
    )
    return _epilogue(v_mean, inputs)



# revision 14
# speedup vs baseline: 1.9279x; 1.9279x over previous
"""Trainium2 Bass kernel: Encoder_HieStackedCorr (UnCorrVmat_Detail, t_method='uncorr').

Math (per batch b):
    W1 = wn(U1_v, U1_g); W2 = wn(U2_v, U2_g)
    R = relu(V @ W1.T + b1)          [N, LR]
    L = relu(V @ W2.T + b2)          [N, LR]
    d[n] = L[n] . R[n];  dr = 1/sqrt(d + eps)
    s[m] = (N + 1 - dr[m] * (t . R[m])) / N,   t = sum_n dr[n] L[n,:]
    v = s @ V ;  feat = v @ W_lin.T + b_lin ; out = batchnorm(feat)

The N x N correlation matrix is never materialized (mean-pool commutes with
the matmul).  This version keeps L/R in *natural* [n, lr] layout via PE
transposes so every per-n vector (d, dr, s) lives across 128 partitions:
rsqrt/affine run as tiny [128, 16] column ops instead of [1, N] row ops.

Layouts per core (4 batches):
    w12_sb [128, 2*128]   lhsT chunks: W12 = [W2; W1] stacked, K=d-chunk
    vt4    [128, 4b*2c*2048n]  V^T  (matmul rhs for L|R)
    v4     [128, 16j*2c*4b*128d]  natural V (rhs for v = s^T V)
    LR4    [128, 16j*4b*128lr]    relu'd stacked [L|R], natural layout
    d4/drf4/u4 ... [128, 4b*16j] f32 column vectors
    s4b    [128, 16j*4b] bf16  (lhsT columns for the final matmul)

Sharding: data-parallel over batch, 4 batches per core on 8 cores.  The tiny
[32,256] linear + batchnorm epilogue (cross-core batch stats) runs on host.
"""

import os
import sys

import numpy as np

for _p in ("/opt/trn_rl_repo", "/root/.axon_site/_ro/trn_rl_repo"):
    if os.path.isdir(_p) and _p not in sys.path:
        sys.path.insert(0, _p)
        break

import ml_dtypes  # noqa: E402
import concourse.bass as bass  # noqa: E402
import concourse.bacc as bacc  # noqa: E402
import concourse.mybir as mybir  # noqa: E402
import concourse.tile as tile  # noqa: E402
from concourse.bass_utils import run_bass_kernel_spmd  # noqa: E402


def _ensure_ntff_hook():
    """Shim the missing ``antenv.axon_hooks`` registry so trace=True works."""
    import types

    try:
        from antenv.axon_hooks import get_axon_ntff_profile_hook  # noqa: F401
        return
    except ImportError:
        pass
    try:
        from trn_agent_boot.trn_boot import _ntff_profile_via_ctypes
        hook = _ntff_profile_via_ctypes("/opt/axon/libaxon_pjrt.so")
    except Exception:
        hook = None
    mod = types.ModuleType("antenv.axon_hooks")
    mod._hook = hook
    mod.get_axon_ntff_profile_hook = lambda: mod._hook
    mod.set_axon_ntff_profile_hook = lambda h: setattr(mod, "_hook", h)
    sys.modules["antenv.axon_hooks"] = mod


_ensure_ntff_hook()

# Problem constants (hardcoded).
B, N, D, LR, EMB = 32, 2048, 256, 64, 256
NCORES = 8
B_LOC = B // NCORES          # 4 batches per core
NT = N // 128                # 16 row-tiles per batch
NBLK = N // 512              # 4 512-row blocks per batch
EPS_DIAG = 1e-6
EPS_BN = 1e-5

F32 = mybir.dt.float32
BF16 = mybir.dt.bfloat16
AF = mybir.ActivationFunctionType
ALU = mybir.AluOpType

CONFIG = dict(trace=False)

_CACHE = {}


def _build():
    nc = bacc.Bacc("TRN2", target_bir_lowering=False, debug=False)

    vt_d = nc.dram_tensor("vt", [2, 128, B_LOC * N], BF16, kind="ExternalInput").ap()
    vn_d = nc.dram_tensor("vn", [128, NT * 2 * B_LOC * 128], BF16,
                          kind="ExternalInput").ap()
    w12_d = nc.dram_tensor("w12", [2, 128, 128], BF16, kind="ExternalInput").ap()
    b12_d = nc.dram_tensor("b12", [128, 1], F32, kind="ExternalInput").ap()
    id_d = nc.dram_tensor("ident", [128, 128], BF16, kind="ExternalInput").ap()
    out_d = nc.dram_tensor("vmean", [1, B_LOC * D], F32, kind="ExternalOutput").ap()

    with tile.TileContext(nc) as tc:
        with (
            tc.tile_pool(name="const", bufs=1) as cpool,
            tc.tile_pool(name="lrt", bufs=2) as lrtpool,
            tc.tile_pool(name="prod", bufs=2) as prpool,
            tc.tile_pool(name="ps_lr", bufs=2, space="PSUM") as ps_lr,
            tc.tile_pool(name="ps_tr", bufs=2, space="PSUM") as ps_tr,
            tc.tile_pool(name="ps_sm", bufs=1, space="PSUM") as ps_sm,
            tc.tile_pool(name="ps_v", bufs=1, space="PSUM") as ps_v,
        ):
            # ---- constants / weights ----
            w12_sb = cpool.tile([128, 2 * 128], BF16)
            nc.sync.dma_start(
                w12_sb[:].rearrange("p (c m) -> p c m", c=2),
                w12_d.rearrange("c p m -> p c m"),
            )
            b12_sb = cpool.tile([128, 1], F32)
            nc.sync.dma_start(b12_sb[:], b12_d[:])
            id_sb = cpool.tile([128, 128], BF16)
            nc.sync.dma_start(id_sb[:], id_d[:])
            eps_sb = cpool.tile([128, 1], F32)
            nc.vector.memset(eps_sb[:], EPS_DIAG)
            ones_k1 = cpool.tile([1, 128], BF16)
            nc.vector.memset(ones_k1[:], 1.0)

            # ---- big persistent tiles ----
            vt4 = cpool.tile([128, B_LOC * 2 * N], BF16)       # [b, c, n]
            v4 = cpool.tile([128, NT * 2 * B_LOC * 128], BF16)  # [j, c, b, d]
            LR4 = cpool.tile([128, NT * B_LOC * 128], BF16)     # [j, b, lr]
            d4 = cpool.tile([128, B_LOC * NT], F32)             # [b, j]
            sq4 = cpool.tile([128, B_LOC * NT], F32)
            drf4 = cpool.tile([128, B_LOC * NT], F32)
            dr4b = cpool.tile([128, B_LOC * NT], BF16)
            u4 = cpool.tile([128, B_LOC * NT], F32)
            cs4 = cpool.tile([128, B_LOC * NT], F32)
            # s columns padded to 32-partition spacing: col 32*b of block j
            # holds s for (j, b); matmul output rows then land at partitions
            # {0,32,64,96}, which compute engines can legally read.
            s4b = cpool.tile([128, NT * 128], BF16)
            t_sb = cpool.tile([1, B_LOC * LR], BF16)
            tb_sb = cpool.tile([128, B_LOC * LR], BF16)

            out32 = cpool.tile([128, D], F32)

            t_ps = ps_sm.tile([1, B_LOC * LR], F32, tag="tps")
            tb_ps = ps_sm.tile([128, B_LOC * LR], F32, tag="tbps")
            v_ps = [ps_v.tile([128, B_LOC * 128], F32, tag=f"vps{c}",
                              name=f"vps{c}")
                    for c in range(2)]

            LR4v = LR4[:].rearrange("p (j b l) -> p j b l", j=NT, b=B_LOC)
            s4bv = s4b[:].rearrange("p (j m) -> p j m", j=NT)
            nc.gpsimd.memset(s4b[:], 0.0)

            # ---- input DMAs (vt per batch; vn in 4 interleaved slabs) ----
            vn_q = NT * 2 * B_LOC * 128 // 4
            for b in range(B_LOC):
                for c in range(2):
                    nc.sync.dma_start(
                        vt4[:, (b * 2 + c) * N:(b * 2 + c + 1) * N],
                        vt_d[c, :, b * N:(b + 1) * N],
                    )
                nc.sync.dma_start(
                    v4[:, b * vn_q:(b + 1) * vn_q],
                    vn_d[:, b * vn_q:(b + 1) * vn_q],
                )

            def emit_blocks(b):
                """L|R for batch b: matmul + relu + transpose to natural."""
                for blk in range(NBLK):
                    lr_ps = ps_lr.tile([128, 512], F32, tag="lrps")
                    for c in range(2):
                        nc.tensor.matmul(
                            lr_ps[:], w12_sb[:, c * 128:(c + 1) * 128],
                            vt4[:, (b * 2 + c) * N + blk * 512:
                                (b * 2 + c) * N + blk * 512 + 512],
                            start=(c == 0), stop=(c == 1),
                        )
                    lrt = lrtpool.tile([128, 512], BF16, tag="lrt")
                    if blk % 2 == 0:
                        nc.scalar.activation(lrt[:], lr_ps[:], AF.Relu,
                                             bias=b12_sb[:], scale=1.0)
                    else:
                        nc.vector.tensor_scalar(lrt[:], lr_ps[:], b12_sb[:],
                                                0.0, ALU.add, ALU.max)
                    tr_ps = ps_tr.tile([128, 512], BF16, tag="trps")
                    for q in range(4):
                        nc.tensor.transpose(
                            tr_ps[:, q * 128:(q + 1) * 128],
                            lrt[:, q * 128:(q + 1) * 128], id_sb[:],
                        )
                    dst = LR4v[:, blk * 4:(blk + 1) * 4, b, :]
                    src = tr_ps[:].rearrange("p (q l) -> p q l", q=4)
                    if blk % 2 == 0:
                        nc.vector.tensor_copy(dst, src)
                    else:
                        nc.scalar.activation(dst, src, AF.Copy)

            def emit_d(b):
                """diag -> dr (column layout) for batch b."""
                Lb = LR4v[:, :, b, 0:LR]
                Rb = LR4v[:, :, b, LR:128]
                pr = prpool.tile([128, NT * LR], BF16, tag="pr")
                prv = pr[:].rearrange("p (j l) -> p j l", j=NT)
                nc.gpsimd.tensor_tensor(prv, Lb, Rb, ALU.mult)
                nc.vector.tensor_reduce(
                    d4[:, b * NT:(b + 1) * NT], prv,
                    mybir.AxisListType.X, ALU.add,
                )
                nc.scalar.activation(
                    sq4[:, b * NT:(b + 1) * NT], d4[:, b * NT:(b + 1) * NT],
                    AF.Sqrt, bias=eps_sb[:], scale=1.0,
                )
                nc.vector.reciprocal(
                    drf4[:, b * NT:(b + 1) * NT], sq4[:, b * NT:(b + 1) * NT]
                )
                nc.scalar.activation(
                    dr4b[:, b * NT:(b + 1) * NT], drf4[:, b * NT:(b + 1) * NT],
                    AF.Copy,
                )

            def emit_t(b):
                """t = sum_n dr[n] L[n,:]; broadcast to 128 partitions."""
                for j in range(NT):
                    nc.tensor.matmul(
                        t_ps[0:1, b * LR:(b + 1) * LR],
                        dr4b[:, b * NT + j:b * NT + j + 1],
                        LR4[:, (j * B_LOC + b) * 128:(j * B_LOC + b) * 128 + LR],
                        start=(j == 0), stop=(j == NT - 1),
                    )
                nc.scalar.activation(
                    t_sb[0:1, b * LR:(b + 1) * LR],
                    t_ps[0:1, b * LR:(b + 1) * LR], AF.Copy,
                )
                nc.tensor.matmul(
                    tb_ps[:, b * LR:(b + 1) * LR], ones_k1[:],
                    t_sb[0:1, b * LR:(b + 1) * LR], start=True, stop=True,
                )
                nc.scalar.activation(
                    tb_sb[:, b * LR:(b + 1) * LR],
                    tb_ps[:, b * LR:(b + 1) * LR], AF.Copy,
                )

            def emit_u(b):
                """u[m] = t . R[m,:] via broadcast multiply + segmented reduce."""
                Rb = LR4v[:, :, b, LR:128]
                tb = tb_sb[:, b * LR:(b + 1) * LR].rearrange(
                    "p (a l) -> p a l", a=1)
                in0, in1 = bass.broadcast_tensor_aps(Rb, tb)
                pr = prpool.tile([128, NT * LR], BF16, tag="pr")
                prv = pr[:].rearrange("p (j l) -> p j l", j=NT)
                nc.gpsimd.tensor_tensor(prv, in0, in1, ALU.mult)
                nc.vector.tensor_reduce(
                    u4[:, b * NT:(b + 1) * NT], prv,
                    mybir.AxisListType.X, ALU.add,
                )

            def emit_cs(b):
                """s = ((N+1) - dr*u)/N, bf16, [j, b] layout."""
                nc.vector.scalar_tensor_tensor(
                    cs4[:, b * NT:(b + 1) * NT], u4[:, b * NT:(b + 1) * NT],
                    -1.0 / N, drf4[:, b * NT:(b + 1) * NT],
                    ALU.mult, ALU.mult,
                )
                nc.scalar.activation(
                    s4bv[:, :, 32 * b], cs4[:, b * NT:(b + 1) * NT],
                    AF.Copy, bias=float(N + 1) / N, scale=1.0,
                )

            def emit_v():
                """v = s^T V, accumulated over the 16 row-tiles; extract diag."""
                v4v = v4[:].rearrange("p (j c b e) -> p j c b e",
                                      j=NT, c=2, b=B_LOC)
                for j in range(NT):
                    for c in range(2):
                        nc.tensor.matmul(
                            v_ps[c][:],
                            s4b[:, j * 128:(j + 1) * 128],
                            v4v[:, j, c, :, :],
                            start=(j == 0), stop=(j == NT - 1),
                        )
                # row 32*b of v_ps[c] holds batch b; pick its diag block.
                for b in range(B_LOC):
                    for c in range(2):
                        nc.scalar.activation(
                            out32[32 * b:32 * b + 1, c * 128:(c + 1) * 128],
                            v_ps[c][32 * b:32 * b + 1, b * 128:(b + 1) * 128],
                            AF.Copy,
                        )
                for b in range(B_LOC):
                    nc.sync.dma_start(
                        out_d[0:1, b * D:(b + 1) * D],
                        out32[32 * b:32 * b + 1, :],
                    )

            # ---- emission order: keep PE busy while vector chains run ----
            emit_blocks(0)
            emit_d(0)
            emit_blocks(1)
            emit_t(0)
            emit_u(0)
            emit_d(1)
            emit_blocks(2)
            emit_cs(0)
            emit_t(1)
            emit_u(1)
            emit_d(2)
            emit_blocks(3)
            emit_cs(1)
            emit_t(2)
            emit_u(2)
            emit_d(3)
            emit_cs(2)
            emit_t(3)
            emit_u(3)
            emit_cs(3)
            emit_v()

    nc.compile()
    return nc


def _host_prep(inputs):
    """Weight-norm, transposes, casts; returns per-core input maps."""
    bf = ml_dtypes.bfloat16

    def wn(v, g):
        return v * (g / np.linalg.norm(v.astype(np.float64), axis=1)).astype(
            np.float32
        )[:, None]

    W1 = wn(np.asarray(inputs["U1_v"], np.float32), np.asarray(inputs["U1_g"], np.float32))
    W2 = wn(np.asarray(inputs["U2_v"], np.float32), np.asarray(inputs["U2_g"], np.float32))
    W12 = np.concatenate([W2, W1], axis=0)  # [128, 256]
    w12 = np.stack([np.ascontiguousarray(W12[:, :128].T),
                    np.ascontiguousarray(W12[:, 128:].T)]).astype(bf)
    b12 = np.concatenate([np.asarray(inputs["U2_b"], np.float32),
                          np.asarray(inputs["U1_b"], np.float32)]).reshape(128, 1)
    ident = np.eye(128, dtype=bf)

    V = np.asarray(inputs["Vmat"], np.float32)  # [B, N, D]
    in_maps = []
    for k in range(NCORES):
        Vk = V[k * B_LOC:(k + 1) * B_LOC]  # [4, 2048, 256]
        vt = np.ascontiguousarray(Vk.transpose(2, 0, 1)).reshape(
            2, 128, B_LOC * N).astype(bf)
        vn = np.ascontiguousarray(
            Vk.reshape(B_LOC, NT, 128, 2, 128).transpose(2, 1, 3, 0, 4)
        ).reshape(128, NT * 2 * B_LOC * 128).astype(bf)
        in_maps.append({
            "vt": np.ascontiguousarray(vt),
            "vn": np.ascontiguousarray(vn),
            "w12": w12,
            "b12": b12,
            "ident": ident,
        })
    return in_maps


def _epilogue(v_mean, inputs):
    """feat = v_mean @ W_lin.T + b_lin, then training-mode batchnorm."""
    W_lin = np.asarray(inputs["W_lin"], np.float32)
    b_lin = np.asarray(inputs["b_lin"], np.float32)
    gamma = np.asarray(inputs["gamma"], np.float32)
    beta = np.asarray(inputs["beta"], np.float32)
    feat = v_mean.astype(np.float32) @ W_lin.T + b_lin
    mu = feat.mean(axis=0)
    var = feat.var(axis=0)
    out = (feat - mu) / np.sqrt(var + EPS_BN) * gamma + beta
    return out.astype(np.float32)


def kernel(**inputs):
    if "nc" not in _CACHE:
        _CACHE["nc"] = _build()
    nc = _CACHE["nc"]
    in_maps = _host_prep(inputs)
    res = run_bass_kernel_spmd(
        nc, in_maps, core_ids=list(range(NCORES)), trace=CONFIG["trace"]
    )
    kernel.last_results = res
    v_mean = np.concatenate(
        [res.results[k]["vmean"].reshape(B_LOC, D) for k in range(NCORES)], axis=0
    )
    return _epilogue(v_mean, inputs)


# revision 18
# speedup vs baseline: 2.4105x; 1.2503x over previous
"""Trainium2 Bass kernel: Encoder_HieStackedCorr (UnCorrVmat_Detail, t_method='uncorr').

Math (per batch b):
    W1 = wn(U1_v, U1_g); W2 = wn(U2_v, U2_g)
    R = relu(V @ W1.T + b1)          [N, LR]
    L = relu(V @ W2.T + b2)          [N, LR]
    d[n] = L[n] . R[n];  dr = 1/sqrt(d + eps)
    s[m] = (N + 1 - dr[m] * (t . R[m])) / N,   t = sum_n dr[n] L[n,:]
    v = s @ V ;  feat = v @ W_lin.T + b_lin ; out = batchnorm(feat)

The N x N correlation matrix is never materialized (mean-pool commutes with
the matmul).  L|R are produced directly in *natural* [n, l|r] layout by using
the V^T tiles as the stationary matmul operand, so every per-n vector
(d, dr, s) lives across 128 partitions: rsqrt/affine are tiny [128, 16]
column ops instead of [1, N] row ops, and no transposes or PSUM-drain copies
are needed.

Contractions over n (t and v) use zero-padded stationary tiles whose active
columns sit at 32-partition spacing, because compute engines may only read
partition offsets that are 32-aligned.

Sharding: data-parallel over batch, 4 batches per core on 8 cores.  The tiny
[32,256] linear + batchnorm epilogue (cross-core batch stats) runs on host.
"""

import os
import sys

import numpy as np

for _p in ("/opt/trn_rl_repo", "/root/.axon_site/_ro/trn_rl_repo"):
    if os.path.isdir(_p) and _p not in sys.path:
        sys.path.insert(0, _p)
        break

import ml_dtypes  # noqa: E402
import concourse.bass as bass  # noqa: E402
import concourse.bacc as bacc  # noqa: E402
import concourse.mybir as mybir  # noqa: E402
import concourse.tile as tile  # noqa: E402
from concourse.bass_utils import run_bass_kernel_spmd  # noqa: E402


def _ensure_ntff_hook():
    """Shim the missing ``antenv.axon_hooks`` registry so trace=True works."""
    import types

    try:
        from antenv.axon_hooks import get_axon_ntff_profile_hook  # noqa: F401
        return
    except ImportError:
        pass
    try:
        from trn_agent_boot.trn_boot import _ntff_profile_via_ctypes
        hook = _ntff_profile_via_ctypes("/opt/axon/libaxon_pjrt.so")
    except Exception:
        hook = None
    mod = types.ModuleType("antenv.axon_hooks")
    mod._hook = hook
    mod.get_axon_ntff_profile_hook = lambda: mod._hook
    mod.set_axon_ntff_profile_hook = lambda h: setattr(mod, "_hook", h)
    sys.modules["antenv.axon_hooks"] = mod


_ensure_ntff_hook()

# Problem constants (hardcoded).
B, N, D, LR, EMB = 32, 2048, 256, 64, 256
NCORES = 8
B_LOC = B // NCORES          # 4 batches per core
NT = N // 128                # 16 row-tiles per batch
EPS_DIAG = 1e-6
EPS_BN = 1e-5

F32 = mybir.dt.float32
BF16 = mybir.dt.bfloat16
AF = mybir.ActivationFunctionType
ALU = mybir.AluOpType

CONFIG = dict(trace=False)

_CACHE = {}


def _build():
    nc = bacc.Bacc("TRN2", target_bir_lowering=False, debug=False)

    vt_d = nc.dram_tensor("vt", [2, 128, B_LOC * N], BF16, kind="ExternalInput").ap()
    vn_d = nc.dram_tensor("vn", [128, NT * 2 * B_LOC * 128], BF16,
                          kind="ExternalInput").ap()
    w12_d = nc.dram_tensor("w12", [2, 128, 128], BF16, kind="ExternalInput").ap()
    b12_d = nc.dram_tensor("b12", [128, 1], F32, kind="ExternalInput").ap()
    out_d = nc.dram_tensor("vmean", [1, B_LOC * D], F32, kind="ExternalOutput").ap()

    with tile.TileContext(nc) as tc:
        with (
            tc.tile_pool(name="const", bufs=1) as cpool,
            tc.tile_pool(name="prod", bufs=2) as prpool,
            tc.tile_pool(name="ps_lr", bufs=3, space="PSUM") as ps_lr,
            tc.tile_pool(name="ps_sm", bufs=1, space="PSUM") as ps_sm,
            tc.tile_pool(name="ps_v", bufs=1, space="PSUM") as ps_v,
        ):
            # ---- big persistent tiles ----
            vt4 = cpool.tile([128, B_LOC * 2 * N], BF16)        # [b, c, n]
            v4 = cpool.tile([128, NT * 2 * B_LOC * 128], BF16)  # [j, c, b, d]
            LR4 = cpool.tile([128, NT * B_LOC * 128], BF16)     # [j, b, lr]
            d4 = cpool.tile([128, B_LOC * NT], F32)             # [b, j]
            sq4 = cpool.tile([128, B_LOC * NT], F32)
            drf4 = cpool.tile([128, B_LOC * NT], F32)
            u4 = cpool.tile([128, B_LOC * NT], F32)
            cs4 = cpool.tile([128, B_LOC * NT], F32)
            # zero-padded stationaries: active col 32*b of block j
            drp = cpool.tile([128, NT * 128], BF16)
            s4b = cpool.tile([128, NT * 128], BF16)
            t_sb = cpool.tile([1, B_LOC * LR], BF16)
            tb_sb = cpool.tile([128, B_LOC * LR], BF16)
            out32 = cpool.tile([128, D], F32)

            # ---- input DMAs first (vt batch 0 leads; compute needs halves) ----
            for b in range(B_LOC):
                for h in range(2):
                    for c in range(2):
                        nc.sync.dma_start(
                            vt4[:, (b * 2 + c) * N + h * 1024:
                                (b * 2 + c) * N + (h + 1) * 1024],
                            vt_d[c, :, b * N + h * 1024:b * N + (h + 1) * 1024],
                        )
                if b == 0:
                    w12_sb = cpool.tile([128, 2 * 128], BF16)
                    nc.sync.dma_start(
                        w12_sb[:].rearrange("p (c m) -> p c m", c=2),
                        w12_d.rearrange("c p m -> p c m"),
                    )
                    b12_sb = cpool.tile([128, 1], F32)
                    nc.sync.dma_start(b12_sb[:], b12_d[:])
            vn_q = NT * 2 * B_LOC * 128 // 4
            for q in range(4):
                nc.sync.dma_start(
                    v4[:, q * vn_q:(q + 1) * vn_q],
                    vn_d[:, q * vn_q:(q + 1) * vn_q],
                )

            eps_sb = cpool.tile([128, 1], F32)
            nc.vector.memset(eps_sb[:], EPS_DIAG)
            ones_k1 = cpool.tile([1, 128], BF16)
            nc.vector.memset(ones_k1[:], 1.0)
            nc.gpsimd.memset(drp[:], 0.0)
            nc.gpsimd.memset(s4b[:], 0.0)

            t_ps = ps_sm.tile([128, B_LOC * 128], F32, tag="tps")
            tb_ps = ps_sm.tile([128, B_LOC * LR], F32, tag="tbps")
            v_ps = [ps_v.tile([128, B_LOC * 128], F32, tag=f"vps{c}",
                              name=f"vps{c}")
                    for c in range(2)]

            LR4v = LR4[:].rearrange("p (j b l) -> p j b l", j=NT, b=B_LOC)
            drpv = drp[:].rearrange("p (j m) -> p j m", j=NT)
            s4bv = s4b[:].rearrange("p (j m) -> p j m", j=NT)

            def emit_blocks(b):
                """L|R for batch b directly in natural [n, l|r] layout."""
                for g in range(4):
                    lr_ps = ps_lr.tile([128, 512], F32, tag="lrps")
                    for q in range(4):
                        j = g * 4 + q
                        for c in range(2):
                            nc.tensor.matmul(
                                lr_ps[:, q * 128:(q + 1) * 128],
                                vt4[:, (b * 2 + c) * N + j * 128:
                                    (b * 2 + c) * N + (j + 1) * 128],
                                w12_sb[:, c * 128:(c + 1) * 128],
                                start=(c == 0), stop=(c == 1),
                            )
                    dst = LR4v[:, g * 4:(g + 1) * 4, b, :]
                    srcv = lr_ps[:].rearrange("p (q l) -> p q l", q=4)
                    nc.scalar.activation(dst, srcv, AF.Relu,
                                         bias=b12_sb[:], scale=1.0)

            def emit_d(b):
                """diag -> dr (column layout) for batch b; dr lands in drp."""
                Lb = LR4v[:, :, b, 0:LR]
                Rb = LR4v[:, :, b, LR:128]
                pr = prpool.tile([128, NT * LR], BF16, tag="pr")
                prv = pr[:].rearrange("p (j l) -> p j l", j=NT)
                nc.vector.tensor_tensor(prv, Lb, Rb, ALU.mult)
                nc.vector.tensor_reduce(
                    d4[:, b * NT:(b + 1) * NT], prv,
                    mybir.AxisListType.X, ALU.add,
                )
                nc.scalar.activation(
                    sq4[:, b * NT:(b + 1) * NT], d4[:, b * NT:(b + 1) * NT],
                    AF.Sqrt, bias=eps_sb[:], scale=1.0,
                )
                nc.vector.reciprocal(
                    drf4[:, b * NT:(b + 1) * NT], sq4[:, b * NT:(b + 1) * NT]
                )
                nc.scalar.activation(
                    drpv[:, :, 32 * b], drf4[:, b * NT:(b + 1) * NT], AF.Copy,
                )

            def emit_t_chain(pair):
                """One chain computes t for every batch whose dr column is
                already in drp; extract/broadcast the two new batches."""
                for j in range(NT):
                    nc.tensor.matmul(
                        t_ps[:], drp[:, j * 128:(j + 1) * 128],
                        LR4[:, j * 512:(j + 1) * 512],
                        start=(j == 0), stop=(j == NT - 1),
                    )
                for b in (2 * pair, 2 * pair + 1):
                    nc.scalar.activation(
                        t_sb[0:1, b * LR:(b + 1) * LR],
                        t_ps[32 * b:32 * b + 1, b * 128:b * 128 + LR], AF.Copy,
                    )
                    nc.tensor.matmul(
                        tb_ps[:, b * LR:(b + 1) * LR], ones_k1[:],
                        t_sb[0:1, b * LR:(b + 1) * LR], start=True, stop=True,
                    )
                    nc.scalar.activation(
                        tb_sb[:, b * LR:(b + 1) * LR],
                        tb_ps[:, b * LR:(b + 1) * LR], AF.Copy,
                    )

            def emit_u(b):
                """u[m] = t . R[m,:] via broadcast multiply + segmented reduce."""
                Rb = LR4v[:, :, b, LR:128]
                tb = tb_sb[:, b * LR:(b + 1) * LR].rearrange(
                    "p (a l) -> p a l", a=1)
                in0, in1 = bass.broadcast_tensor_aps(Rb, tb)
                pr = prpool.tile([128, NT * LR], BF16, tag="pr")
                prv = pr[:].rearrange("p (j l) -> p j l", j=NT)
                nc.vector.tensor_tensor(prv, in0, in1, ALU.mult)
                nc.vector.tensor_reduce(
                    u4[:, b * NT:(b + 1) * NT], prv,
                    mybir.AxisListType.X, ALU.add,
                )

            def emit_cs(b):
                """s = ((N+1) - dr*u)/N, bf16, padded column 32*b."""
                nc.vector.scalar_tensor_tensor(
                    cs4[:, b * NT:(b + 1) * NT], u4[:, b * NT:(b + 1) * NT],
                    -1.0 / N, drf4[:, b * NT:(b + 1) * NT],
                    ALU.mult, ALU.mult,
                )
                nc.scalar.activation(
                    s4bv[:, :, 32 * b], cs4[:, b * NT:(b + 1) * NT],
                    AF.Copy, bias=float(N + 1) / N, scale=1.0,
                )

            def emit_v():
                """v = s^T V accumulated over row-tiles; batch b at row 32*b."""
                v4v = v4[:].rearrange("p (j c b e) -> p j c b e",
                                      j=NT, c=2, b=B_LOC)
                for j in range(NT):
                    for c in range(2):
                        nc.tensor.matmul(
                            v_ps[c][:],
                            s4b[:, j * 128:(j + 1) * 128],
                            v4v[:, j, c, :, :],
                            start=(j == 0), stop=(j == NT - 1),
                        )
                for b in range(B_LOC):
                    for c in range(2):
                        src = v_ps[c][32 * b:32 * b + 1,
                                      b * 128:(b + 1) * 128]
                        dst = out32[32 * b:32 * b + 1, c * 128:(c + 1) * 128]
                        if c == 0:
                            nc.scalar.activation(dst, src, AF.Copy)
                        else:
                            nc.vector.tensor_copy(dst, src)
                for b in range(B_LOC):
                    nc.sync.dma_start(
                        out_d[0:1, b * D:(b + 1) * D],
                        out32[32 * b:32 * b + 1, :],
                    )

            # ---- emission order: keep PE busy while vector chains run ----
            emit_blocks(0)
            emit_d(0)
            emit_blocks(1)
            emit_d(1)
            emit_blocks(2)
            emit_t_chain(0)
            emit_u(0)
            emit_cs(0)
            emit_u(1)
            emit_d(2)
            emit_blocks(3)
            emit_cs(1)
            emit_d(3)
            emit_t_chain(1)
            emit_u(2)
            emit_cs(2)
            emit_u(3)
            emit_cs(3)
            emit_v()

    nc.compile()
    return nc


def _host_prep(inputs):
    """Weight-norm, transposes, casts; returns per-core input maps."""
    bf = ml_dtypes.bfloat16

    def wn(v, g):
        return v * (g / np.linalg.norm(v.astype(np.float64), axis=1)).astype(
            np.float32
        )[:, None]

    W1 = wn(np.asarray(inputs["U1_v"], np.float32), np.asarray(inputs["U1_g"], np.float32))
    W2 = wn(np.asarray(inputs["U2_v"], np.float32), np.asarray(inputs["U2_g"], np.float32))
    W12 = np.concatenate([W2, W1], axis=0)  # [128, 256]
    w12 = np.stack([np.ascontiguousarray(W12[:, :128].T),
                    np.ascontiguousarray(W12[:, 128:].T)]).astype(bf)
    b12 = np.concatenate([np.asarray(inputs["U2_b"], np.float32),
                          np.asarray(inputs["U1_b"], np.float32)]).reshape(128, 1)

    V = np.asarray(inputs["Vmat"], np.float32)  # [B, N, D]
    in_maps = []
    for k in range(NCORES):
        Vk = V[k * B_LOC:(k + 1) * B_LOC]  # [4, 2048, 256]
        vt = np.ascontiguousarray(Vk.transpose(2, 0, 1)).reshape(
            2, 128, B_LOC * N).astype(bf)
        vn = np.ascontiguousarray(
            Vk.reshape(B_LOC, NT, 128, 2, 128).transpose(2, 1, 3, 0, 4)
        ).reshape(128, NT * 2 * B_LOC * 128).astype(bf)
        in_maps.append({
            "vt": np.ascontiguousarray(vt),
            "vn": np.ascontiguousarray(vn),
            "w12": w12,
            "b12": b12,
        })
    return in_maps


def _epilogue(v_mean, inputs):
    """feat = v_mean @ W_lin.T + b_lin, then training-mode batchnorm."""
    W_lin = np.asarray(inputs["W_lin"], np.float32)
    b_lin = np.asarray(inputs["b_lin"], np.float32)
    gamma = np.asarray(inputs["gamma"], np.float32)
    beta = np.asarray(inputs["beta"], np.float32)
    feat = v_mean.astype(np.float32) @ W_lin.T + b_lin
    mu = feat.mean(axis=0)
    var = feat.var(axis=0)
    out = (feat - mu) / np.sqrt(var + EPS_BN) * gamma + beta
    return out.astype(np.float32)


def kernel(**inputs):
    if "nc" not in _CACHE:
        _CACHE["nc"] = _build()
    nc = _CACHE["nc"]
    in_maps = _host_prep(inputs)
    res = run_bass_kernel_spmd(
        nc, in_maps, core_ids=list(range(NCORES)), trace=CONFIG["trace"]
    )
    kernel.last_results = res
    v_mean = np.concatenate(
        [res.results[k]["vmean"].reshape(B_LOC, D) for k in range(NCORES)], axis=0
    )
    return _epilogue(v_mean, inputs)
